# revision 36
# baseline (speedup 1.0000x reference)
"""CrossAttnBlock kernel for 8x Trainium2 NeuronCores.

Problem (hardcoded shapes): x,target [8,256,64,64] f32; GroupNorm(32 groups) on
both; q = Wq@gn(x), k = Wk@gn(t), v = Wv@gn(t) (1x1 convs); softmax cross
attention over HW=4096 pixels; out = Wp@(attn) + bp.

Sharding: data-parallel over batch B=8 -> one batch per core. Within a core the
whole block is computed in channel-major layout [C=256, HW=4096]:

  scores are built TRANSPOSED: sT[m,n] = sum_c k[c,m] q[c,n] via
  matmul(lhsT=k_tile, rhs=q_tile) so no on-chip transposes are ever needed.
  pT = exp(sT/16) directly (max-free softmax: scores are ~N(0,1), exp is safe).
  h_unnorm[c,n] = sum_m v_pm[m,c] pT[m,n]  (lhsT = pixel-major v, produced
  pixel-major straight from the projection matmul).
  softmax denominators accumulate on the otherwise-idle DVE (acc += pT), and
  the 1/sum plus the +bp bias are folded in after the (linear) output
  projection:  out[o,n] = (Wp @ h_unnorm)[o,n] * recip[n] + (Wp@bv + bp)[o]
  where the (Wp@bv+bp) row rides the final matmul as an extra channel
  multiplied by sum[n], so the recip multiply finishes both terms at once.

The attention inner loop is software-pipelined (scores(mt) ahead of PV(mt-1))
so the in-order PE queue never stalls behind exp; chunk tails are deferred
into the next chunk's loop. Heavy matmuls run in float32r (1 cycle/row on
TRN2 vs 4 for fp32), ~1.6e-4 relative error per 256-deep contraction.
"""
import numpy as np

import concourse.bacc as bacc
import concourse.bass as bass
import concourse.mybir as mybir
import concourse.tile as tile
from concourse.bass_utils import run_bass_kernel_spmd

F32 = mybir.dt.float32
F32R = mybir.dt.float32r
BF16 = mybir.dt.bfloat16
AF = mybir.ActivationFunctionType
ALU = mybir.AluOpType

B, C, H, W = 8, 256, 64, 64
HW = H * W            # 4096
G = 32                # groups
EPS = 1e-5
NCH = 8               # n-chunks of 512 query pixels
NC512 = HW // NCH     # 512
MT = HW // 128        # 32 key tiles
LCH = 4               # load/apply chunking per c-tile
LSZ = HW // LCH       # 1024
SCALE = C ** -0.5     # 1/16


def _build_program():
    nc = bacc.Bacc("TRN2", target_bir_lowering=False)

    x_d = nc.dram_tensor("x", [2, 128, HW], BF16, kind="ExternalInput")
    t_d = nc.dram_tensor("t", [2, 128, HW], BF16, kind="ExternalInput")
    w_d = {}
    for nm in ("wq", "wk", "wv"):
        w_d[nm] = nc.dram_tensor(nm, [2, 128, C], BF16, kind="ExternalInput")
    w_d["wp"] = nc.dram_tensor("wp", [2, 128, C], F32, kind="ExternalInput")
    b_d = {}
    for nm in ("bq", "bk", "bv", "bp", "gs", "gb"):
        b_d[nm] = nc.dram_tensor(nm, [2, 128, 1], F32, kind="ExternalInput")
    gsel_d = nc.dram_tensor("gsel", [2, 128, G], F32, kind="ExternalInput")
    gexp_d = nc.dram_tensor("gexp", [2, G, 128], F32, kind="ExternalInput")
    out_d = nc.dram_tensor("out", [2, 128, HW], BF16, kind="ExternalOutput")

    with tile.TileContext(nc) as tc:
        with (
            tc.tile_pool(name="big", bufs=1) as big,
            tc.tile_pool(name="wgt", bufs=1) as wgt,
            tc.tile_pool(name="sm", bufs=1) as sm,
            tc.tile_pool(name="pt", bufs=4) as ptp,
            tc.tile_pool(name="tail", bufs=1) as tailp,
        ):
            ps_setup = tc.alloc_tile_pool(name="ps_setup", bufs=2, space="PSUM")
            # ---- loads: t first (critical), biases, weights (f32r direct), x
            xin_y = big.tile([128, 2, HW], BF16, tag="in_y", name="in_y")
            xin_x = big.tile([128, 2, HW], BF16, tag="in_x", name="in_x")
            for i in range(2):
                nc.sync.dma_start(out=xin_y[:, i, :], in_=t_d[i, :, :])
            b_sb = {}
            for nm in ("bq", "bk", "bv", "bp", "gs", "gb"):
                b_sb[nm] = sm.tile([128, 2], F32, tag=f"b_{nm}", name=f"b_{nm}")
                nc.sync.dma_start(out=b_sb[nm], in_=b_d[nm][:].rearrange("i p o -> p i o"))
            gsel_sb = sm.tile([128, 2, G], F32)
            nc.sync.dma_start(out=gsel_sb, in_=gsel_d[:].rearrange("i p g -> p i g"))
            gexp_sb = sm.tile([32, 2, 128], F32)
            nc.sync.dma_start(out=gexp_sb, in_=gexp_d[:].rearrange("i g c -> g i c"))
            # x before the weights: GN-x stats gate the first q projection;
            # weights aren't consumed until ~20us in.
            for i in range(2):
                nc.sync.dma_start(out=xin_x[:, i, :], in_=x_d[i, :, :])
            # q/k/v weights arrive bf16 and are DMAed straight into their
            # compute tiles (no staging copy); wp stays f32 for the output
            # projection's f32r precision.
            w_r = {}
            for nm in ("wk", "wv", "wq"):
                w_r[nm] = wgt.tile([128, 2, C], BF16, tag=f"{nm}_r", name=f"{nm}_r")
                nc.sync.dma_start(out=w_r[nm], in_=w_d[nm][:].rearrange("i p o -> p i o"))
            wp_st = wgt.tile([128, 2, C], F32, tag="wp_st", name="wp_st")
            nc.sync.dma_start(out=wp_st, in_=w_d["wp"][:].rearrange("i p o -> p i o"))
            eps_t = sm.tile([128, 1], F32)
            nc.vector.memset(eps_t, EPS)

            # ---- group norm: stats on DVE; the cross-partition group
            # combine and per-channel expansion ride tiny fp32 matmuls on the
            # (idle at startup) PE instead of latency-bound scatter DMAs.
            def group_norm(xin, tag, out_tag):
                hout = big.tile([128, 2, HW], BF16, tag=out_tag, name=f"gn_{tag}")
                ps_gsum = ps_setup.tile([G, 1], F32, tag="ps_gn", name=f"ps_gsum_{tag}", bufs=2)
                ps_gmsq = ps_setup.tile([G, 1], F32, tag="ps_gn", name=f"ps_gmsq_{tag}", bufs=2)
                mvs = []
                for i in range(2):
                    stats = sm.tile([128, 8, 6], F32, tag="bn_st", name=f"bnst_{tag}{i}")
                    xg = xin[:, i, :].rearrange("p (s f) -> p s f", f=512)
                    for s in range(8):
                        nc.vector.bn_stats(out=stats[:, s, :], in_=xg[:, s, :])
                    mv = sm.tile([128, 2], F32, tag=f"bn_mv{i}", name=f"bnmv_{tag}{i}")
                    nc.vector.bn_aggr(out=mv, in_=stats)
                    msq = sm.tile([128, 1], F32, tag=f"bn_msq{i}", name=f"bnmsq_{tag}{i}")
                    nc.vector.tensor_mul(msq, mv[:, 0:1], mv[:, 0:1])
                    nc.vector.tensor_add(msq, msq, mv[:, 1:2])
                    nc.tensor.matmul(ps_gsum, gsel_sb[:, i, :], mv[:, 0:1],
                                     start=(i == 0), stop=(i == 1))
                    nc.tensor.matmul(ps_gmsq, gsel_sb[:, i, :], msq,
                                     start=(i == 0), stop=(i == 1))
                gmean = sm.tile([G, 1], F32, tag="gmean", name=f"gmean_{tag}")
                nc.vector.tensor_scalar_mul(gmean, ps_gsum, 1.0 / 8.0)
                gvar = sm.tile([G, 1], F32, tag="gvar", name=f"gvar_{tag}")
                nc.vector.tensor_scalar_mul(gvar, ps_gmsq, 1.0 / 8.0)
                gms = sm.tile([G, 1], F32, tag="gms", name=f"gms_{tag}")
                nc.vector.tensor_mul(gms, gmean, gmean)
                nc.vector.tensor_sub(gvar, gvar, gms)
                nc.scalar.activation(gvar, gvar, AF.Sqrt, bias=eps_t[0:G, :])
                nc.vector.reciprocal(gvar, gvar)          # rstd per group
                for i in range(2):
                    ps_rstd = ps_setup.tile([128, 1], F32, tag="ps_gn2", name=f"ps_rstd_{tag}{i}", bufs=2)
                    ps_mean = ps_setup.tile([128, 1], F32, tag="ps_gn2", name=f"ps_mean_{tag}{i}", bufs=2)
                    nc.tensor.matmul(ps_rstd, gexp_sb[:, i, :], gvar, start=True, stop=True)
                    nc.tensor.matmul(ps_mean, gexp_sb[:, i, :], gmean, start=True, stop=True)
                    alpha = sm.tile([128, 1], F32, tag="alpha", name=f"alpha_{tag}{i}")
                    beta = sm.tile([128, 1], F32, tag="beta", name=f"beta_{tag}{i}")
                    nc.vector.tensor_mul(alpha, ps_rstd, b_sb["gs"][:, i:i + 1])
                    nc.vector.tensor_mul(beta, ps_mean, alpha)
                    nc.vector.tensor_sub(beta, b_sb["gb"][:, i:i + 1], beta)
                    for cth in range(LCH):
                        csl = slice(cth * LSZ, (cth + 1) * LSZ)
                        nc.scalar.activation(hout[:, i, csl], xin[:, i, csl],
                                             AF.Identity, bias=beta, scale=alpha)
                return hout

            # target side first: k and v unblock the attention pipeline
            hy = group_norm(xin_y, "y", out_tag="gn_y")
            ones_st = sm.tile([128, 128], F32)
            nc.gpsimd.memset(ones_st, 1.0)
            ones_blk = sm.tile([128, 128], F32R)   # partition-reduction lhsT
            nc.gpsimd.tensor_copy(ones_blk, ones_st)

            # ---- projections: emitted one 512-pixel chunk at a time so the
            # attention loop can start as soon as k/q chunk 0 exist; later k
            # chunks, all v tiles, and q chunks 1-7 are produced just-in-time
            # inside the attention loop, filling otherwise-idle PE slots.
            def proj_chunk(dst, wname, bname, src_gn, nch, pool, tag):
                nsl = slice(nch * NC512, (nch + 1) * NC512)
                for j in range(2):
                    ps_p = pool.tile([128, NC512], F32, tag=tag,
                                     name=f"ps_{wname}", bufs=1)
                    for i in range(2):
                        nc.tensor.matmul(ps_p, w_r[wname][:, i, j * 128:(j + 1) * 128],
                                         src_gn[:, i, nsl], start=(i == 0), stop=(i == 1))
                    nc.scalar.activation(dst[:, j, nsl], ps_p, AF.Identity,
                                         bias=b_sb[bname][:, j:j + 1])

            k_r = big.tile([128, 2, HW], F32R, tag="k", name="k_r")
            proj_chunk(k_r, "wk", "bk", hy, 0, ps_setup, "ps_proj")
            hx = group_norm(xin_x, "x", out_tag="gn_x")
            # v (pixel-major) on the PE while the DVE runs GN-x stats; the
            # PSUM->SBUF copies ride the Pool engine.
            v_r = big.tile([128, MT, C], F32R, tag="v", name="v_r")
            for mt in range(MT):
                msl = slice(mt * 128, (mt + 1) * 128)
                ps_v = ps_setup.tile([128, C], F32, tag="ps_v", name="ps_v", bufs=3)
                for i in range(2):
                    nc.tensor.matmul(ps_v, hy[:, i, msl], w_r["wv"][:, i, :],
                                     start=(i == 0), stop=(i == 1))
                nc.vector.tensor_copy(v_r[:, mt, :], ps_v)
            q_r = big.tile([128, 2, HW], F32R, tag="q", name="q_r")

            # bias row for the final projection: bpp = Wp @ bv + bp -> [1,256] f32r
            w_r["wp"] = wgt.tile([128, 2, C], F32R, tag="wp_r", name="wp_r")
            nc.vector.tensor_copy(w_r["wp"], wp_st)
            bpp_f32 = sm.tile([1, C], F32)
            for j in range(2):
                ps_bp = ps_setup.tile([128, 1], F32, tag="ps_gn2", name="ps_bp", bufs=2)
                for i in range(2):
                    nc.tensor.matmul(ps_bp, wp_st[:, i, j * 128:(j + 1) * 128],
                                     b_sb["bv"][:, i:i + 1], start=(i == 0), stop=(i == 1))
                bp_col = sm.tile([128, 1], F32, tag="bp_col", name="bp_col")
                nc.scalar.activation(bp_col, ps_bp, AF.Identity, bias=b_sb["bp"][:, j:j + 1])
                nc.gpsimd.dma_start(out=bpp_f32[0:1, j * 128:(j + 1) * 128], in_=bp_col)
            bpp_row = sm.tile([1, C], F32R)
            nc.vector.tensor_copy(bpp_row, bpp_f32)

            ps_setup.release()
            # PSUM plan (8 banks): ps_sc x2, ps_h0 x2, ps_h1 x2, ps_aux (jit
            # q/k projections + denominator reduction), ps_o.
            ps = tc.alloc_tile_pool(name="ps_att", bufs=1, space="PSUM")
            ps_s = tc.alloc_tile_pool(name="ps_sc2", bufs=2, space="PSUM")
            # ---- attention -----------------------------------------------
            # software-pipelined: scores(mt) issue ahead of PV(mt-1) so the
            # in-order PE queue never stalls behind exp. Chunk 0 additionally
            # produces k chunks 1-7 just-in-time; every chunk produces its
            # own q chunk first. Each chunk's tail (h copies + output
            # projection) is deferred into the next chunk's loop.
            deferred_tail = None
            deferred_den = None
            for nch in range(NCH):
                nsl = slice(nch * NC512, (nch + 1) * NC512)
                if nch == 0:
                    proj_chunk(q_r, "wq", "bq", hx, 0, ps, "ps_aux")
                ps_h0 = ps.tile([128, NC512], F32, tag="ps_h0", name="ps_h0", bufs=2)
                ps_h1 = ps.tile([128, NC512], F32, tag="ps_h1", name="ps_h1", bufs=2)
                # denominator accumulates on two engines in parallel: even key
                # tiles on the DVE, odd ones on the (otherwise idle) Pool
                # engine; merged by the ones-matmul partition reduction below.
                acc_d = tailp.tile([128, NC512], F32, tag="acc_d", name="acc_d")
                acc_p = tailp.tile([128, NC512], F32, tag="acc_p", name="acc_p")
                pts = [None] * MT
                SKEW = 2          # exp(mt) has 2 full iterations to complete
                for mt in range(MT + SKEW):
                    if nch == 0 and mt % 4 == 0 and mt < 28:
                        # jit k projection, 4 key-tiles ahead of consumption
                        proj_chunk(k_r, "wk", "bk", hy, mt // 4 + 1, ps, "ps_aux")
                    if mt == (18 if nch == 0 else 16) and nch + 1 < NCH:
                        # prefetch next chunk's q mid-chunk, while the shared
                        # aux PSUM bank is guaranteed idle
                        proj_chunk(q_r, "wq", "bq", hx, nch + 1, ps, "ps_aux")
                    if mt < MT:
                        msl = slice(mt * 128, (mt + 1) * 128)
                        ps_sc = ps_s.tile([128, NC512], F32, tag="ps_sc", name="ps_sc")
                        nc.tensor.matmul(ps_sc, k_r[:, 0, msl], q_r[:, 0, nsl], start=True, stop=False)
                        nc.tensor.matmul(ps_sc, k_r[:, 1, msl], q_r[:, 1, nsl], start=False, stop=True)
                        pT = ptp.tile([128, NC512], F32R, tag="pT", name="pT")
                        nc.scalar.activation(pT, ps_sc, AF.Exp, scale=SCALE)
                        pts[mt] = pT
                    if mt == 1 and deferred_den is not None:
                        deferred_den()
                        deferred_den = None
                    if mt == 3 and deferred_tail is not None:
                        deferred_tail()
                        deferred_tail = None
                    if mt >= SKEW:
                        pv = pts[mt - SKEW]
                        st, sp = (mt - SKEW == 0), (mt - SKEW == MT - 1)
                        nc.tensor.matmul(ps_h0, v_r[:, mt - SKEW, 0:128], pv, start=st, stop=sp)
                        nc.tensor.matmul(ps_h1, v_r[:, mt - SKEW, 128:256], pv, start=st, stop=sp)
                        # softmax denominator: alternate DVE / Pool engines
                        j = mt - SKEW
                        if j == 0:
                            nc.vector.tensor_copy(acc_d, pv)
                        elif j == 1:
                            nc.gpsimd.tensor_copy(acc_p, pv)
                        elif j % 2 == 0:
                            nc.vector.tensor_add(acc_d, acc_d, pv)
                        else:
                            nc.gpsimd.tensor_add(acc_p, acc_p, pv)
                # finish the denominator: acc_d/acc_p hold per-partition
                # partial sums (16 tiles each); two ones-matmuls reduce the
                # 128 partitions and merge both accumulators in PSUM. This is
                # deferred to mt==1 of the next chunk so the PE's wait on the
                # trailing accumulator copies overlaps the next scores.
                acc_dr = tailp.tile([128, NC512], F32R, tag="acc_dr", name="acc_dr")
                acc_pr = tailp.tile([128, NC512], F32R, tag="acc_pr", name="acc_pr")
                recipb = tailp.tile([128, NC512], F32, tag="recipb", name="recipb")
                hs = tailp.tile([1, NC512], F32R, tag="hs", name="hs")

                def make_den(acc_d=acc_d, acc_p=acc_p, acc_dr=acc_dr,
                             acc_pr=acc_pr, recipb=recipb, hs=hs):
                    def den():
                        nc.vector.tensor_copy(acc_dr, acc_d)
                        nc.gpsimd.tensor_copy(acc_pr, acc_p)
                        ps_sum = ps.tile([128, NC512], F32, tag="ps_aux", name="ps_sum", bufs=1)
                        nc.tensor.matmul(ps_sum, ones_blk, acc_dr, start=True, stop=False)
                        nc.tensor.matmul(ps_sum, ones_blk, acc_pr, start=False, stop=True)
                        nc.vector.reciprocal(recipb, ps_sum)
                        nc.vector.tensor_copy(hs, ps_sum[0:1, :])
                    return den

                deferred_den = make_den()

                def make_tail(nsl=nsl, ps_h0=ps_h0, ps_h1=ps_h1, recipb=recipb, hs=hs):
                    def tail():
                        h0 = tailp.tile([128, NC512], F32R, tag="h0", name="h0")
                        h1 = tailp.tile([128, NC512], F32R, tag="h1", name="h1")
                        nc.vector.tensor_copy(h0, ps_h0)
                        nc.vector.tensor_copy(h1, ps_h1)
                        for j in range(2):
                            osl = slice(j * 128, (j + 1) * 128)
                            ps_o = ps.tile([128, NC512], F32, tag="ps_o", name="ps_o", bufs=1)
                            nc.tensor.matmul(ps_o, w_r["wp"][:, 0, osl], h0, start=True, stop=False)
                            nc.tensor.matmul(ps_o, w_r["wp"][:, 1, osl], h1, start=False, stop=False)
                            nc.tensor.matmul(ps_o, bpp_row[:, osl], hs, start=False, stop=True)
                            o_sb = tailp.tile([128, NC512], BF16, tag="o_sb", name="o_sb", bufs=2)
                            nc.vector.tensor_mul(o_sb, ps_o, recipb)
                            nc.sync.dma_start(out=out_d[j, :, nsl], in_=o_sb)
                    return tail

                deferred_tail = make_tail()
            deferred_den()
            deferred_tail()
            ps_s.release()
            ps.release()
    nc.compile()
    return nc


_prog = None


def kernel(**inputs):
    global _prog
    import ml_dtypes
    bf16 = ml_dtypes.bfloat16
    x = np.ascontiguousarray(np.asarray(inputs["x"], np.float32).astype(bf16))
    t = np.ascontiguousarray(np.asarray(inputs["target"], np.float32).astype(bf16))
    gs = np.asarray(inputs["gn_scale"], np.float32)
    gb = np.asarray(inputs["gn_bias"], np.float32)
    Ws = {nm: np.ascontiguousarray(np.asarray(inputs[k], np.float32).T.reshape(2, 128, C)
                                   .astype(bf16 if nm != "wp" else np.float32))
          for nm, k in (("wq", "Wq"), ("wk", "Wk"), ("wv", "Wv"), ("wp", "Wp"))}
    bs = {nm: np.ascontiguousarray(np.asarray(inputs[k], np.float32).reshape(2, 128, 1))
          for nm, k in (("bq", "bq"), ("bk", "bk"), ("bv", "bv"), ("bp", "bp"))}
    bs["gs"] = np.ascontiguousarray(gs.reshape(2, 128, 1))
    bs["gb"] = np.ascontiguousarray(gb.reshape(2, 128, 1))
    cc = np.arange(128)[:, None] // 8
    gg = np.arange(G)[None, :]
    gsel = np.stack([(cc + 16 * i == gg).astype(np.float32) for i in range(2)])
    bs["gsel"] = np.ascontiguousarray(gsel)                      # [2,128,G]
    bs["gexp"] = np.ascontiguousarray(gsel.transpose(0, 2, 1))   # [2,G,128]

    if _prog is None:
        _prog = _build_program()

    in_maps = []
    for b in range(B):
        m = {"x": x[b].reshape(2, 128, HW), "t": t[b].reshape(2, 128, HW)}
        m.update(Ws)
        m.update(bs)
        in_maps.append(m)
    res = run_bass_kernel_spmd(_prog, in_maps, core_ids=list(range(B)))
    out = np.stack([r["out"].astype(np.float32).reshape(C, H, W) for r in res.results])
    return out



# revision 37
# speedup vs baseline: 1.0314x; 1.0314x over previous
"""CrossAttnBlock kernel for 8x Trainium2 NeuronCores.

Problem (hardcoded shapes): x,target [8,256,64,64] f32; GroupNorm(32 groups) on
both; q = Wq@gn(x), k = Wk@gn(t), v = Wv@gn(t) (1x1 convs); softmax cross
attention over HW=4096 pixels; out = Wp@(attn) + bp.

Sharding: data-parallel over batch B=8 -> one batch per core. Within a core the
whole block is computed in channel-major layout [C=256, HW=4096]:

  scores are built TRANSPOSED: sT[m,n] = sum_c k[c,m] q[c,n] via
  matmul(lhsT=k_tile, rhs=q_tile) so no on-chip transposes are ever needed.
  pT = exp(sT/16) directly (max-free softmax: scores are ~N(0,1), exp is safe).
  h_unnorm[c,n] = sum_m v_pm[m,c] pT[m,n]  (lhsT = pixel-major v, produced
  pixel-major straight from the projection matmul).
  softmax denominators accumulate split across the DVE and Pool engines
  (even/odd key tiles), and the 1/sum plus the +bp bias are folded in after
  the (linear) output projection:
  out[o,n] = (Wp @ h_unnorm)[o,n] * recip[n] + (Wp@bv + bp)[o]
  where the (Wp@bv+bp) row rides the final matmul as an extra channel
  multiplied by sum[n], so the recip multiply finishes both terms at once.

x/target/Wq/Wk/Wv are staged in bf16 (half the HBM + host-tunnel traffic; the
inputs are the real DMA bottleneck), GN outputs and q/k/v projections run as
pure-bf16 matmuls (same 1 cycle/row as f32r on TRN2), while scores/PV/output
projection stay f32r and all accumulation is fp32 PSUM / f32 SBUF.
End-to-end max-abs relative error ~6e-3 against the fp32 reference.

The attention inner loop is software-pipelined (scores(mt) ahead of PV(mt-1))
so the in-order PE queue never stalls behind exp; chunk tails and the
denominator reduction are deferred into the next chunk's loop; q projections
are produced per-chunk just-in-time (prefetched mid-previous-chunk) and k
chunks 1-7 are produced inside chunk 0, so attention starts ~30us in instead
of waiting for the full setup phase.
"""
import numpy as np

import concourse.bacc as bacc
import concourse.bass as bass
import concourse.mybir as mybir
import concourse.tile as tile
from concourse.bass_utils import run_bass_kernel_spmd

F32 = mybir.dt.float32
F32R = mybir.dt.float32r
BF16 = mybir.dt.bfloat16
AF = mybir.ActivationFunctionType
ALU = mybir.AluOpType

B, C, H, W = 8, 256, 64, 64
HW = H * W            # 4096
G = 32                # groups
EPS = 1e-5
NCH = 8               # n-chunks of 512 query pixels
NC512 = HW // NCH     # 512
MT = HW // 128        # 32 key tiles
LCH = 4               # load/apply chunking per c-tile
LSZ = HW // LCH       # 1024
SCALE = C ** -0.5     # 1/16


def _build_program():
    nc = bacc.Bacc("TRN2", target_bir_lowering=False)

    x_d = nc.dram_tensor("x", [2, 128, HW], BF16, kind="ExternalInput")
    t_d = nc.dram_tensor("t", [2, 128, HW], BF16, kind="ExternalInput")
    w_d = {}
    for nm in ("wq", "wk", "wv"):
        w_d[nm] = nc.dram_tensor(nm, [2, 128, C], BF16, kind="ExternalInput")
    w_d["wp"] = nc.dram_tensor("wp", [2, 128, C], F32, kind="ExternalInput")
    b_d = {}
    for nm in ("bq", "bk", "bv", "bp", "gs", "gb"):
        b_d[nm] = nc.dram_tensor(nm, [2, 128, 1], F32, kind="ExternalInput")
    gsel_d = nc.dram_tensor("gsel", [2, 128, G], F32, kind="ExternalInput")
    gexp_d = nc.dram_tensor("gexp", [2, G, 128], F32, kind="ExternalInput")
    out_d = nc.dram_tensor("out", [2, 128, HW], BF16, kind="ExternalOutput")

    with tile.TileContext(nc) as tc:
        with (
            tc.tile_pool(name="big", bufs=1) as big,
            tc.tile_pool(name="wgt", bufs=1) as wgt,
            tc.tile_pool(name="sm", bufs=1) as sm,
            tc.tile_pool(name="pt", bufs=4) as ptp,
            tc.tile_pool(name="tail", bufs=1) as tailp,
        ):
            ps_setup = tc.alloc_tile_pool(name="ps_setup", bufs=2, space="PSUM")
            # ---- loads: t first (critical), biases, weights (f32r direct), x
            xin_y = big.tile([128, 2, HW], BF16, tag="in_y", name="in_y")
            xin_x = big.tile([128, 2, HW], BF16, tag="in_x", name="in_x")
            for i in range(2):
                nc.sync.dma_start(out=xin_y[:, i, :], in_=t_d[i, :, :])
            b_sb = {}
            for nm in ("bq", "bk", "bv", "bp", "gs", "gb"):
                b_sb[nm] = sm.tile([128, 2], F32, tag=f"b_{nm}", name=f"b_{nm}")
                nc.sync.dma_start(out=b_sb[nm], in_=b_d[nm][:].rearrange("i p o -> p i o"))
            gsel_sb = sm.tile([128, 2, G], F32)
            nc.sync.dma_start(out=gsel_sb, in_=gsel_d[:].rearrange("i p g -> p i g"))
            gexp_sb = sm.tile([32, 2, 128], F32)
            nc.sync.dma_start(out=gexp_sb, in_=gexp_d[:].rearrange("i g c -> g i c"))
            # x before the weights: GN-x stats gate the first q projection;
            # weights aren't consumed until ~20us in.
            for i in range(2):
                nc.sync.dma_start(out=xin_x[:, i, :], in_=x_d[i, :, :])
            # q/k/v weights arrive bf16 and are DMAed straight into their
            # compute tiles (no staging copy); wp stays f32 for the output
            # projection's f32r precision.
            w_r = {}
            for nm in ("wk", "wv", "wq"):
                w_r[nm] = wgt.tile([128, 2, C], BF16, tag=f"{nm}_r", name=f"{nm}_r")
                nc.sync.dma_start(out=w_r[nm], in_=w_d[nm][:].rearrange("i p o -> p i o"))
            wp_st = wgt.tile([128, 2, C], F32, tag="wp_st", name="wp_st")
            nc.sync.dma_start(out=wp_st, in_=w_d["wp"][:].rearrange("i p o -> p i o"))
            eps_t = sm.tile([128, 1], F32)
            nc.vector.memset(eps_t, EPS)

            # ---- group norm: stats on DVE; the cross-partition group
            # combine and per-channel expansion ride tiny fp32 matmuls on the
            # (idle at startup) PE instead of latency-bound scatter DMAs.
            def group_norm(xin, tag, out_tag):
                hout = big.tile([128, 2, HW], BF16, tag=out_tag, name=f"gn_{tag}")
                ps_gsum = ps_setup.tile([G, 1], F32, tag="ps_gn", name=f"ps_gsum_{tag}", bufs=2)
                ps_gmsq = ps_setup.tile([G, 1], F32, tag="ps_gn", name=f"ps_gmsq_{tag}", bufs=2)
                mvs = []
                for i in range(2):
                    stats = sm.tile([128, 8, 6], F32, tag="bn_st", name=f"bnst_{tag}{i}")
                    xg = xin[:, i, :].rearrange("p (s f) -> p s f", f=512)
                    for s in range(8):
                        nc.vector.bn_stats(out=stats[:, s, :], in_=xg[:, s, :])
                    mv = sm.tile([128, 2], F32, tag=f"bn_mv{i}", name=f"bnmv_{tag}{i}")
                    nc.vector.bn_aggr(out=mv, in_=stats)
                    msq = sm.tile([128, 1], F32, tag=f"bn_msq{i}", name=f"bnmsq_{tag}{i}")
                    nc.vector.tensor_mul(msq, mv[:, 0:1], mv[:, 0:1])
                    nc.vector.tensor_add(msq, msq, mv[:, 1:2])
                    nc.tensor.matmul(ps_gsum, gsel_sb[:, i, :], mv[:, 0:1],
                                     start=(i == 0), stop=(i == 1))
                    nc.tensor.matmul(ps_gmsq, gsel_sb[:, i, :], msq,
                                     start=(i == 0), stop=(i == 1))
                gmean = sm.tile([G, 1], F32, tag="gmean", name=f"gmean_{tag}")
                nc.vector.tensor_scalar_mul(gmean, ps_gsum, 1.0 / 8.0)
                gvar = sm.tile([G, 1], F32, tag="gvar", name=f"gvar_{tag}")
                nc.vector.tensor_scalar_mul(gvar, ps_gmsq, 1.0 / 8.0)
                gms = sm.tile([G, 1], F32, tag="gms", name=f"gms_{tag}")
                nc.vector.tensor_mul(gms, gmean, gmean)
                nc.vector.tensor_sub(gvar, gvar, gms)
                nc.scalar.activation(gvar, gvar, AF.Sqrt, bias=eps_t[0:G, :])
                nc.vector.reciprocal(gvar, gvar)          # rstd per group
                for i in range(2):
                    ps_rstd = ps_setup.tile([128, 1], F32, tag="ps_gn2", name=f"ps_rstd_{tag}{i}", bufs=2)
                    ps_mean = ps_setup.tile([128, 1], F32, tag="ps_gn2", name=f"ps_mean_{tag}{i}", bufs=2)
                    nc.tensor.matmul(ps_rstd, gexp_sb[:, i, :], gvar, start=True, stop=True)
                    nc.tensor.matmul(ps_mean, gexp_sb[:, i, :], gmean, start=True, stop=True)
                    alpha = sm.tile([128, 1], F32, tag="alpha", name=f"alpha_{tag}{i}")
                    beta = sm.tile([128, 1], F32, tag="beta", name=f"beta_{tag}{i}")
                    nc.vector.tensor_mul(alpha, ps_rstd, b_sb["gs"][:, i:i + 1])
                    nc.vector.tensor_mul(beta, ps_mean, alpha)
                    nc.vector.tensor_sub(beta, b_sb["gb"][:, i:i + 1], beta)
                    for cth in range(LCH):
                        csl = slice(cth * LSZ, (cth + 1) * LSZ)
                        nc.scalar.activation(hout[:, i, csl], xin[:, i, csl],
                                             AF.Identity, bias=beta, scale=alpha)
                return hout

            # target side first: k and v unblock the attention pipeline
            hy = group_norm(xin_y, "y", out_tag="gn_y")
            ones_st = sm.tile([128, 128], F32)
            nc.gpsimd.memset(ones_st, 1.0)
            ones_blk = sm.tile([128, 128], F32R)   # partition-reduction lhsT
            nc.gpsimd.tensor_copy(ones_blk, ones_st)

            # ---- projections: emitted one 512-pixel chunk at a time so the
            # attention loop can start as soon as k/q chunk 0 exist; later k
            # chunks, all v tiles, and q chunks 1-7 are produced just-in-time
            # inside the attention loop, filling otherwise-idle PE slots.
            def proj_chunk(dst, wname, bname, src_gn, nch, pool, tag):
                nsl = slice(nch * NC512, (nch + 1) * NC512)
                for j in range(2):
                    ps_p = pool.tile([128, NC512], F32, tag=tag,
                                     name=f"ps_{wname}", bufs=1)
                    for i in range(2):
                        nc.tensor.matmul(ps_p, w_r[wname][:, i, j * 128:(j + 1) * 128],
                                         src_gn[:, i, nsl], start=(i == 0), stop=(i == 1))
                    nc.scalar.activation(dst[:, j, nsl], ps_p, AF.Identity,
                                         bias=b_sb[bname][:, j:j + 1])

            k_r = big.tile([128, 2, HW], F32R, tag="k", name="k_r")
            proj_chunk(k_r, "wk", "bk", hy, 0, ps_setup, "ps_proj")
            hx = group_norm(xin_x, "x", out_tag="gn_x")
            # v (pixel-major) on the PE while the DVE runs GN-x stats; the
            # PSUM->SBUF copies ride the Pool engine.
            v_r = big.tile([128, MT, C], F32R, tag="v", name="v_r")
            for mt in range(MT):
                msl = slice(mt * 128, (mt + 1) * 128)
                ps_v = ps_setup.tile([128, C], F32, tag="ps_v", name="ps_v", bufs=3)
                for i in range(2):
                    nc.tensor.matmul(ps_v, hy[:, i, msl], w_r["wv"][:, i, :],
                                     start=(i == 0), stop=(i == 1))
                nc.vector.tensor_copy(v_r[:, mt, :], ps_v)
            q_r = big.tile([128, 2, HW], F32R, tag="q", name="q_r")

            # bias row for the final projection: bpp = Wp @ bv + bp -> [1,256] f32r
            w_r["wp"] = wgt.tile([128, 2, C], F32R, tag="wp_r", name="wp_r")
            nc.vector.tensor_copy(w_r["wp"], wp_st)
            bpp_f32 = sm.tile([1, C], F32)
            for j in range(2):
                ps_bp = ps_setup.tile([128, 1], F32, tag="ps_gn2", name="ps_bp", bufs=2)
                for i in range(2):
                    nc.tensor.matmul(ps_bp, wp_st[:, i, j * 128:(j + 1) * 128],
                                     b_sb["bv"][:, i:i + 1], start=(i == 0), stop=(i == 1))
                bp_col = sm.tile([128, 1], F32, tag="bp_col", name="bp_col")
                nc.scalar.activation(bp_col, ps_bp, AF.Identity, bias=b_sb["bp"][:, j:j + 1])
                nc.gpsimd.dma_start(out=bpp_f32[0:1, j * 128:(j + 1) * 128], in_=bp_col)
            bpp_row = sm.tile([1, C], F32R)
            nc.vector.tensor_copy(bpp_row, bpp_f32)

            ps_setup.release()
            # PSUM plan (8 banks): ps_sc x2, ps_h0 x2, ps_h1 x2, ps_aux (jit
            # q/k projections + denominator reduction), ps_o.
            ps = tc.alloc_tile_pool(name="ps_att", bufs=1, space="PSUM")
            ps_s = tc.alloc_tile_pool(name="ps_sc2", bufs=2, space="PSUM")
            # ---- attention -----------------------------------------------
            # software-pipelined: scores(mt) issue ahead of PV(mt-1) so the
            # in-order PE queue never stalls behind exp. Chunk 0 additionally
            # produces k chunks 1-7 just-in-time; every chunk produces its
            # own q chunk first. Each chunk's tail (h copies + output
            # projection) is deferred into the next chunk's loop.
            deferred_tail = None
            deferred_den = None
            for nch in range(NCH):
                nsl = slice(nch * NC512, (nch + 1) * NC512)
                if nch == 0:
                    proj_chunk(q_r, "wq", "bq", hx, 0, ps, "ps_aux")
                ps_h0 = ps.tile([128, NC512], F32, tag="ps_h0", name="ps_h0", bufs=2)
                ps_h1 = ps.tile([128, NC512], F32, tag="ps_h1", name="ps_h1", bufs=2)
                # denominator accumulates on two engines in parallel: even key
                # tiles on the DVE, odd ones on the (otherwise idle) Pool
                # engine; merged by the ones-matmul partition reduction below.
                acc_d = tailp.tile([128, NC512], F32, tag="acc_d", name="acc_d")
                acc_p = tailp.tile([128, NC512], F32, tag="acc_p", name="acc_p")
                pts = [None] * MT
                SKEW = 2          # exp(mt) has 2 full iterations to complete
                for mt in range(MT + SKEW):
                    if nch == 0 and mt % 4 == 0 and mt < 28:
                        # jit k projection, 4 key-tiles ahead of consumption
                        proj_chunk(k_r, "wk", "bk", hy, mt // 4 + 1, ps, "ps_aux")
                    if mt == (18 if nch == 0 else 16) and nch + 1 < NCH:
                        # prefetch next chunk's q mid-chunk, while the shared
                        # aux PSUM bank is guaranteed idle
                        proj_chunk(q_r, "wq", "bq", hx, nch + 1, ps, "ps_aux")
                    if mt < MT:
                        msl = slice(mt * 128, (mt + 1) * 128)
                        ps_sc = ps_s.tile([128, NC512], F32, tag="ps_sc", name="ps_sc")
                        nc.tensor.matmul(ps_sc, k_r[:, 0, msl], q_r[:, 0, nsl], start=True, stop=False)
                        nc.tensor.matmul(ps_sc, k_r[:, 1, msl], q_r[:, 1, nsl], start=False, stop=True)
                        pT = ptp.tile([128, NC512], F32R, tag="pT", name="pT")
                        nc.scalar.activation(pT, ps_sc, AF.Exp, scale=SCALE)
                        pts[mt] = pT
                    if mt == 1 and deferred_den is not None:
                        deferred_den()
                        deferred_den = None
                    if mt == 3 and deferred_tail is not None:
                        deferred_tail()
                        deferred_tail = None
                    if mt >= SKEW:
                        pv = pts[mt - SKEW]
                        st, sp = (mt - SKEW == 0), (mt - SKEW == MT - 1)
                        nc.tensor.matmul(ps_h0, v_r[:, mt - SKEW, 0:128], pv, start=st, stop=sp)
                        nc.tensor.matmul(ps_h1, v_r[:, mt - SKEW, 128:256], pv, start=st, stop=sp)
                        # softmax denominator: alternate DVE / Pool engines
                        j = mt - SKEW
                        if j == 0:
                            nc.vector.tensor_copy(acc_d, pv)
                        elif j == 1:
                            nc.gpsimd.tensor_copy(acc_p, pv)
                        elif j % 2 == 0:
                            nc.vector.tensor_add(acc_d, acc_d, pv)
                        else:
                            nc.gpsimd.tensor_add(acc_p, acc_p, pv)
                # finish the denominator: acc_d/acc_p hold per-partition
                # partial sums (16 tiles each); two ones-matmuls reduce the
                # 128 partitions and merge both accumulators in PSUM. This is
                # deferred to mt==1 of the next chunk so the PE's wait on the
                # trailing accumulator copies overlaps the next scores.
                acc_dr = tailp.tile([128, NC512], F32R, tag="acc_dr", name="acc_dr")
                acc_pr = tailp.tile([128, NC512], F32R, tag="acc_pr", name="acc_pr")
                recipb = tailp.tile([128, NC512], F32, tag="recipb", name="recipb")
                hs = tailp.tile([1, NC512], F32R, tag="hs", name="hs")

                def make_den(acc_d=acc_d, acc_p=acc_p, acc_dr=acc_dr,
                             acc_pr=acc_pr, recipb=recipb, hs=hs):
                    def den():
                        nc.vector.tensor_copy(acc_dr, acc_d)
                        nc.gpsimd.tensor_copy(acc_pr, acc_p)
                        ps_sum = ps.tile([128, NC512], F32, tag="ps_aux", name="ps_sum", bufs=1)
                        nc.tensor.matmul(ps_sum, ones_blk, acc_dr, start=True, stop=False)
                        nc.tensor.matmul(ps_sum, ones_blk, acc_pr, start=False, stop=True)
                        nc.vector.reciprocal(recipb, ps_sum)
                        nc.vector.tensor_copy(hs, ps_sum[0:1, :])
                    return den

                deferred_den = make_den()

                def make_tail(nsl=nsl, ps_h0=ps_h0, ps_h1=ps_h1, recipb=recipb, hs=hs):
                    def tail():
                        h0 = tailp.tile([128, NC512], F32R, tag="h0", name="h0")
                        h1 = tailp.tile([128, NC512], F32R, tag="h1", name="h1")
                        nc.vector.tensor_copy(h0, ps_h0)
                        nc.vector.tensor_copy(h1, ps_h1)
                        for j in range(2):
                            osl = slice(j * 128, (j + 1) * 128)
                            ps_o = ps.tile([128, NC512], F32, tag="ps_o", name="ps_o", bufs=1)
                            nc.tensor.matmul(ps_o, w_r["wp"][:, 0, osl], h0, start=True, stop=False)
                            nc.tensor.matmul(ps_o, w_r["wp"][:, 1, osl], h1, start=False, stop=False)
                            nc.tensor.matmul(ps_o, bpp_row[:, osl], hs, start=False, stop=True)
                            o_sb = tailp.tile([128, NC512], BF16, tag="o_sb", name="o_sb", bufs=2)
                            nc.vector.tensor_mul(o_sb, ps_o, recipb)
                            nc.sync.dma_start(out=out_d[j, :, nsl], in_=o_sb)
                    return tail

                deferred_tail = make_tail()
            deferred_den()
            deferred_tail()
            ps_s.release()
            ps.release()
    nc.compile()
    return nc


_prog = None


def kernel(**inputs):
    global _prog
    import ml_dtypes
    bf16 = ml_dtypes.bfloat16
    x = np.ascontiguousarray(np.asarray(inputs["x"], np.float32).astype(bf16))
    t = np.ascontiguousarray(np.asarray(inputs["target"], np.float32).astype(bf16))
    gs = np.asarray(inputs["gn_scale"], np.float32)
    gb = np.asarray(inputs["gn_bias"], np.float32)
    Ws = {nm: np.ascontiguousarray(np.asarray(inputs[k], np.float32).T.reshape(2, 128, C)
                                   .astype(bf16 if nm != "wp" else np.float32))
          for nm, k in (("wq", "Wq"), ("wk", "Wk"), ("wv", "Wv"), ("wp", "Wp"))}
    bs = {nm: np.ascontiguousarray(np.asarray(inputs[k], np.float32).reshape(2, 128, 1))
          for nm, k in (("bq", "bq"), ("bk", "bk"), ("bv", "bv"), ("bp", "bp"))}
    bs["gs"] = np.ascontiguousarray(gs.reshape(2, 128, 1))
    bs["gb"] = np.ascontiguousarray(gb.reshape(2, 128, 1))
    cc = np.arange(128)[:, None] // 8
    gg = np.arange(G)[None, :]
    gsel = np.stack([(cc + 16 * i == gg).astype(np.float32) for i in range(2)])
    bs["gsel"] = np.ascontiguousarray(gsel)                      # [2,128,G]
    bs["gexp"] = np.ascontiguousarray(gsel.transpose(0, 2, 1))   # [2,G,128]

    if _prog is None:
        _prog = _build_program()

    in_maps = []
    for b in range(B):
        m = {"x": x[b].reshape(2, 128, HW), "t": t[b].reshape(2, 128, HW)}
        m.update(Ws)
        m.update(bs)
        in_maps.append(m)
    res = run_bass_kernel_spmd(_prog, in_maps, core_ids=list(range(B)))
    out = np.stack([r["out"].astype(np.float32).reshape(C, H, W) for r in res.results])
    return out

